# revision 20
# baseline (speedup 1.0000x reference)
"""Trainium2 Bass kernel for DisturbanceRegressionLoss2Heads.

Reference computation (per batch element b, per pixel (h, w)):
  y0 = out[b, 0]  (Y=30 time steps)   y1 = out[b, 1]
  diff = [-7, 0, y0[2]-y0[1], ..., y0[28]-y0[27], 0]
  d = argmin(diff)  (first min)
  piecewise OLS fit of y0 over t<d (x=t) and t>=d (x=t-d), slopes clipped to
  [0,2] in the fitted line, intercepts clipped to [0,100]
  loss = mean over everything of (fitted - y1)^2

Strategy: pure data parallel over the batch (8 cores, one batch element each).
Per core, pixels live on [128 partitions x 512 free]; the 30-step time axis is
in the free dimension.  y0 and y1 are each DMA'd once into full-core resident
tiles (no DMA destination reuse -> no cross-queue WAW sync-slot pressure) and
processed in 4 chunks of 128 pixels/partition: the argmin is one running-min
tensor_tensor_scan over pixel-major diffs with multiply-by-zero boundary
resets; mask(t<d) falls out as (running_min != final_min); segment sums come
from masked products + innermost-axis reduces; a per-pixel OLS epilogue forms
clipped slopes/intercepts; the piecewise-fitted curve is assembled with
broadcast APs + copy_predicated and the squared residual is accumulated
per-partition by the scalar engine's activation(Square, accum_out).
Each core writes 128 x NCHUNK partial sums; the host sums them in float64.
"""

import numpy as np

import concourse.bass as bass
import concourse.bacc as bacc
import concourse.tile as tile
from concourse import mybir
from concourse.bass_utils import run_bass_kernel_spmd

F32 = mybir.dt.float32
BF16 = mybir.dt.bfloat16
AX = mybir.AxisListType
OP = mybir.AluOpType
AF = mybir.ActivationFunctionType

B = 8
Y = 30
H = 256
W = 256
NPIX = H * W          # 65536 pixels per core
P = 128               # SBUF partitions
FP = NPIX // P        # 512 pixels per partition, whole core
F = 128               # pixels per partition per chunk
NCHUNK = FP // F      # 4
DIST = 7.0
MAXI = 100.0


def _emit_chunk(nc, pools, c, bigA, bigB, z, tb, partial):
    """One chunk: argmin, masked sums, OLS epilogue, fitted curve, residual."""
    work, sm = pools
    cs = c * F

    y0pt = bigA[:, :, cs:cs + F].rearrange("p t f -> p f t")  # [P, F, Y] view
    y1pt = bigB[:, :, cs:cs + F].rearrange("p t f -> p f t")

    def b3(small):  # broadcast a [P, F] per-pixel tile along t
        return small[:][:, :, None].broadcast_to([P, F, Y])

    # ---- modified diff array, pixel-major (t contiguous per pixel)
    dpt = work.tile([P, F, Y], F32, tag="dpt")
    nc.vector.tensor_tensor(
        out=dpt[:, :, 2:29], in0=y0pt[:, :, 2:29], in1=y0pt[:, :, 1:28],
        op=OP.subtract)
    nc.vector.memset(dpt[:, :, 0:1], -DIST)
    nc.vector.memset(dpt[:, :, 1:2], 0.0)
    nc.vector.memset(dpt[:, :, 29:30], 0.0)

    # ---- running min along t (reset at pixel boundaries via z=0 slots)
    M = work.tile([P, F, Y], F32, tag="M")
    nc.vector.tensor_tensor_scan(
        out=M[:].rearrange("p f t -> p (f t)"),
        data0=z[:].rearrange("p f t -> p (f t)"),
        data1=dpt[:].rearrange("p f t -> p (f t)"),
        initial=0.0, op0=OP.mult, op1=OP.min)

    # ---- final min per pixel, then maskB = [t < d] = (runmin != finalmin)
    mst = sm.tile([P, F], F32, tag="mst")
    nc.vector.tensor_copy(mst[:], M[:, :, Y - 1])
    maskB = work.tile([P, F, Y], F32, tag="maskB")
    nc.vector.tensor_tensor(out=maskB[:], in0=M[:], in1=b3(mst),
                            op=OP.not_equal)

    # ---- d (= n_before) and masked/unmasked first-order sums over t
    d = sm.tile([P, F], F32, tag="d")
    nc.vector.tensor_reduce(out=d[:], in_=maskB[:], axis=AX.X, op=OP.add)

    pb = work.tile([P, F, Y], F32, tag="dpt")     # reuse dpt slot group
    nc.vector.tensor_tensor(out=pb[:], in0=maskB[:], in1=y0pt, op=OP.mult)
    syb = sm.tile([P, F], F32, tag="syb")
    nc.vector.tensor_reduce(out=syb[:], in_=pb[:], axis=AX.X, op=OP.add)

    # t-weighted masked sum: multiply pb by t in place (gpsimd), then reduce
    # (2D corner self-copy first: as a *write* to pb it inherits both the
    # RAW dep on pb's writer and the WAR dep on the syb reduce, so the
    # DVE-sem wait lands on this sync-roomy copy instead of the 3D op.
    # Chunk 0 schedules differently and still ends up over the sync-slot
    # budget on the 3D op, so it runs on DVE where all deps are same-engine.)
    if c == 0:
        nc.vector.tensor_tensor(out=pb[:], in0=pb[:], in1=tb, op=OP.mult)
    else:
        nc.gpsimd.tensor_copy(pb[:, 0, 0:1], pb[:, 0, 0:1])
        nc.gpsimd.tensor_tensor(out=pb[:], in0=pb[:], in1=tb, op=OP.mult)
    styb = sm.tile([P, F], F32, tag="styb")
    nc.vector.tensor_reduce(out=styb[:], in_=pb[:], axis=AX.X, op=OP.add)

    # totals: reduce y0, then scale y0 by t in place (its last use), reduce
    ty = sm.tile([P, F], F32, tag="ty")
    nc.vector.tensor_reduce(out=ty[:], in_=y0pt, axis=AX.X, op=OP.add)
    if c == 0:
        nc.vector.tensor_tensor(out=y0pt, in0=y0pt, in1=tb, op=OP.mult)
    else:
        nc.gpsimd.tensor_copy(y0pt[:, 0, 0:1], y0pt[:, 0, 0:1])
        nc.gpsimd.tensor_tensor(out=y0pt, in0=y0pt, in1=tb, op=OP.mult)
    tty = sm.tile([P, F], F32, tag="tty")
    nc.vector.tensor_reduce(out=tty[:], in_=y0pt, axis=AX.X, op=OP.add)

    # ---- per-pixel regression epilogue ([P, F] smalls)
    def tt(name, a, bb, op):
        t = sm.tile([P, F], F32, tag=name)
        nc.vector.tensor_tensor(out=t[:], in0=a[:], in1=bb[:], op=op)
        return t

    na = sm.tile([P, F], F32, tag="na")           # 30 - d
    nc.vector.tensor_scalar(out=na[:], in0=d[:], scalar1=-1.0, scalar2=float(Y),
                            op0=OP.mult, op1=OP.add)
    sya = tt("sya", ty, syb, OP.subtract)         # sum y, t>=d
    t0 = tt("t0", tty, styb, OP.subtract)         # sum t*y, t>=d
    t1 = tt("t1", d, sya, OP.mult)
    nc.vector.tensor_tensor(out=t0[:], in0=t0[:], in1=t1[:], op=OP.subtract)
    sxya = t0                                     # sum (t-d)*y, t>=d

    nbs = sm.tile([P, F], F32, tag="nbs")
    nc.vector.tensor_scalar(out=nbs[:], in0=d[:], scalar1=1.0, scalar2=None,
                            op0=OP.max)
    rb = sm.tile([P, F], F32, tag="rb")
    nc.vector.reciprocal(out=rb[:], in_=nbs[:])
    ra = sm.tile([P, F], F32, tag="ra")
    nc.vector.reciprocal(out=ra[:], in_=na[:])

    myb = tt("myb", syb, rb, OP.mult)             # mean y before
    mya = tt("mya", sya, ra, OP.mult)             # mean y after
    mxb = sm.tile([P, F], F32, tag="mxb")         # (d-1)/2
    nc.vector.tensor_scalar(out=mxb[:], in0=d[:], scalar1=1.0, scalar2=0.5,
                            op0=OP.subtract, op1=OP.mult)
    mxa = sm.tile([P, F], F32, tag="mxa")         # (na-1)/2
    nc.vector.tensor_scalar(out=mxa[:], in0=na[:], scalar1=1.0, scalar2=0.5,
                            op0=OP.subtract, op1=OP.mult)

    covb = tt("covb", mxb, syb, OP.mult)          # mxb*syb, then styb - that
    nc.vector.tensor_tensor(out=covb[:], in0=styb[:], in1=covb[:],
                            op=OP.subtract)
    cova = tt("cova", mxa, sya, OP.mult)
    nc.vector.tensor_tensor(out=cova[:], in0=sxya[:], in1=cova[:],
                            op=OP.subtract)

    # var*12 = n*(n^2-1); slope = cov / max(var, 1) gated on var > 0
    vb12 = tt("vb12", d, d, OP.mult)
    nc.vector.scalar_tensor_tensor(out=vb12[:], in0=vb12[:], scalar=1.0,
                                   in1=d[:], op0=OP.subtract, op1=OP.mult)
    mvb = sm.tile([P, F], F32, tag="mvb")
    nc.vector.tensor_scalar(out=mvb[:], in0=vb12[:], scalar1=1.0 / 12.0,
                            scalar2=1.0, op0=OP.mult, op1=OP.max)
    nc.vector.reciprocal(out=mvb[:], in_=mvb[:])
    slb = tt("slb", covb, mvb, OP.mult)
    nc.vector.tensor_scalar(out=vb12[:], in0=vb12[:], scalar1=0.0, scalar2=None,
                            op0=OP.is_gt)        # gate, reuses vb12
    nc.vector.tensor_tensor(out=slb[:], in0=slb[:], in1=vb12[:], op=OP.mult)

    va12 = tt("va12", na, na, OP.mult)
    nc.vector.scalar_tensor_tensor(out=va12[:], in0=va12[:], scalar=1.0,
                                   in1=na[:], op0=OP.subtract, op1=OP.mult)
    nc.vector.tensor_scalar(out=va12[:], in0=va12[:], scalar1=1.0 / 12.0,
                            scalar2=1.0, op0=OP.mult, op1=OP.max)
    nc.vector.reciprocal(out=va12[:], in_=va12[:])
    sla = tt("sla", cova, va12, OP.mult)          # slope after (na>=2 always)

    # intercepts use the *unclipped* slope; fitted lines use clipped slopes
    ibv = tt("ibv", slb, mxb, OP.mult)
    nc.vector.tensor_tensor(out=ibv[:], in0=myb[:], in1=ibv[:], op=OP.subtract)
    nc.vector.tensor_scalar(out=ibv[:], in0=ibv[:], scalar1=0.0, scalar2=MAXI,
                            op0=OP.max, op1=OP.min)
    iav = tt("iav", sla, mxa, OP.mult)
    nc.vector.tensor_tensor(out=iav[:], in0=mya[:], in1=iav[:], op=OP.subtract)
    nc.vector.tensor_scalar(out=iav[:], in0=iav[:], scalar1=0.0, scalar2=MAXI,
                            op0=OP.max, op1=OP.min)
    sbc = sm.tile([P, F], F32, tag="sbc")
    nc.vector.tensor_scalar(out=sbc[:], in0=slb[:], scalar1=0.0, scalar2=2.0,
                            op0=OP.max, op1=OP.min)
    sac = sm.tile([P, F], F32, tag="sac")
    nc.vector.tensor_scalar(out=sac[:], in0=sla[:], scalar1=0.0, scalar2=2.0,
                            op0=OP.max, op1=OP.min)
    ia2f = tt("ia2f", sac, d, OP.mult)            # ia - sac*d
    nc.vector.tensor_tensor(out=ia2f[:], in0=iav[:], in1=ia2f[:],
                            op=OP.subtract)

    # ---- fitted curve: fa = sac*t + ia2f, overwritten with fb = sbc*t + ibv
    # where t < d; then residual vs y1, square + accumulate on scalar engine.
    fa = work.tile([P, F, Y], F32, tag="dpt")
    nc.vector.tensor_tensor(out=fa[:], in0=b3(sac), in1=tb, op=OP.mult)
    nc.vector.tensor_tensor(out=fa[:], in0=fa[:], in1=b3(ia2f), op=OP.add)
    fb = work.tile([P, F, Y], F32, tag="M")
    nc.gpsimd.tensor_tensor(out=fb[:], in0=b3(sbc), in1=tb, op=OP.mult)
    nc.gpsimd.tensor_tensor(out=fb[:], in0=fb[:], in1=b3(ibv), op=OP.add)
    # 2D touch: DVE observes the Pool sem before the predicated copy
    tfb = sm.tile([P, 1], F32, tag="tfb")
    nc.vector.tensor_copy(tfb[:], fb[:, 0, 0:1])
    nc.vector.copy_predicated(out=fa[:], mask=maskB[:].bitcast(mybir.dt.int32),
                              data=fb[:])
    if c == 0:
        # 2D touch: DVE observes y1's DMA-queue sem before the 3D residual op
        tby = sm.tile([P, 1], F32, tag="tby")
        nc.vector.tensor_copy(tby[:], bigB[:, 0, 0:1])
    nc.vector.tensor_tensor(out=fa[:], in0=fa[:], in1=y1pt, op=OP.subtract)
    nc.scalar.activation(out=fa[:], in_=fa[:], func=AF.Square,
                         accum_out=partial[:, c:c + 1])


def build_core_program():
    """Build the per-core Bass program (same program on all 8 cores)."""
    from contextlib import ExitStack

    nc = bacc.Bacc(trn_type="TRN2")
    y0d = nc.dram_tensor("y0", [Y, NPIX], F32, kind="ExternalInput")
    y1d = nc.dram_tensor("y1", [Y, NPIX], F32, kind="ExternalInput")
    outd = nc.dram_tensor("partial", [P, NCHUNK], F32, kind="ExternalOutput")

    with tile.TileContext(nc) as tc, ExitStack() as ctx:
        singles = ctx.enter_context(tc.tile_pool(name="singles", bufs=1))
        io = ctx.enter_context(tc.tile_pool(name="io", bufs=1))
        work = ctx.enter_context(tc.tile_pool(name="work", bufs=1))
        sm = ctx.enter_context(tc.tile_pool(name="sm", bufs=1))

        # constants: z (bf16, 1 except 0 at t=0 of each pixel), t tile (f32)
        z = singles.tile([P, F, Y], BF16)
        nc.vector.memset(z[:], 1.0)
        nc.vector.memset(z[:, :, 0:1], 0.0)
        trow_i = sm.tile([P, Y], mybir.dt.int32, tag="trow_i")
        nc.gpsimd.iota(trow_i[:], pattern=[[1, Y]], base=0, channel_multiplier=0)
        trow = sm.tile([P, Y], F32, tag="trow")
        nc.vector.tensor_copy(trow[:], trow_i[:])
        tvec = singles.tile([P, F, Y], F32)
        nc.vector.tensor_copy(
            tvec[:], trow[:][:, None, :].broadcast_to([P, F, Y]))
        tb = tvec[:]
        partial = singles.tile([P, NCHUNK], F32)

        # resident inputs, one DMA each (no destination reuse)
        bigA = io.tile([P, Y, FP], F32, tag="bigA")
        bigB = io.tile([P, Y, FP], F32, tag="bigB")
        nc.sync.dma_start(out=bigA[:],
                          in_=y0d[:].rearrange("y (p f) -> p y f", p=P))
        nc.sync.dma_start(out=bigB[:],
                          in_=y1d[:].rearrange("y (p f) -> p y f", p=P))

        # 2D touch: gpsimd observes y0's DMA-queue sem before in-place ops
        tga = sm.tile([P, 1], F32, tag="tga")
        nc.gpsimd.tensor_copy(tga[:], bigA[:, 0, 0:1])

        pools = (work, sm)
        for c in range(NCHUNK):
            _emit_chunk(nc, pools, c, bigA[:], bigB[:], z, tb, partial)

        nc.sync.dma_start(out=outd[:, :], in_=partial[:])

    nc.finalize()   # Bacc: runs reg-alloc + the 1-wait sync-split lowering
    return nc


_NC = None


def _get_nc():
    global _NC
    if _NC is None:
        _NC = build_core_program()
    return _NC


def kernel(out, target=None, **_ignored):
    """Full-input entry point: shards batch over 8 cores, returns scalar loss."""
    out = np.ascontiguousarray(out, dtype=np.float32)
    assert out.shape == (B, 2, Y, H, W), out.shape
    nc = _get_nc()
    in_maps = [
        {
            "y0": out[b, 0].reshape(Y, NPIX),
            "y1": out[b, 1].reshape(Y, NPIX),
        }
        for b in range(B)
    ]
    res = run_bass_kernel_spmd(nc, in_maps, core_ids=list(range(B)))
    total = sum(r["partial"].astype(np.float64).sum() for r in res.results)
    loss = total / float(B * Y * NPIX)
    return np.float32(loss)
